# revision 1
# baseline (speedup 1.0000x reference)
"""Trainium2 Bass kernel for nn_NeuralMemory (test-time-training memory layer).

Mathematical reformulation (validated vs the jax reference to ~7e-7):
  * Every per-chunk gradient is taken at the same initial params
    (gamma0, w1_0, w2_0), and the two chunk-axis linear scans (momentum,
    decay) are linear in the gradients.  The final updated weights
    therefore collapse to a single token-weighted backward pass:
        rho_t   = (2/d) * W_{c(t)} * lr_{c(t)}
        W_j     = K_j + eta_{j+1} W_{j+1},   K_j = prod_{i>j} keep_i
        w1_f    = -k^T @ dz,  w2_f = -a^T @ dh,  g_f = -sum_t dpred*hn
    with dpred_t = rho_t * (pred_t - v_t) etc.  No per-chunk grad tensors,
    no associative scans over weight matrices.
  * Retrieval is a plain forward pass with (g_f, w1_f, w2_f).

Sharding: core c handles sample s=c//2; the gradient accumulation (which
needs all 4096 tokens of the sample) is replicated within the core pair,
and retrieval is split (core does tokens [hf*2048,(hf+1)*2048), hf=c%2).
This avoids any cross-core communication (collectives have a ~20us floor
on trn2, larger than the replicated compute).

Layouts: activations flow token-major ([tokens<=128 part, feat free]) for
the normalization/backward elementwise chain (per-token scalars become
native [P,1] tensor_scalar operands) and feature-major where a matmul
needs the contraction on partitions.  The only runtime PE transpose is
dh (one 128x128 per tile).
"""
import numpy as np

import concourse.bass as bass
import concourse.bacc as bacc
import concourse.mybir as mybir
import concourse.tile as tile
from concourse import bass_utils

FP = mybir.dt.float32
AF = mybir.ActivationFunctionType
OP = mybir.AluOpType

B, N, D, HID = 4, 4096, 128, 512
CHUNK = 64
NC = N // CHUNK            # 64 chunks
NT = N // 128              # 32 token-tiles (grad chain)
NRT = (N // 2) // 128      # 16 token-tiles (retrieval half)
NCORES = 8

_CACHED = {}


def _consts():
    FLIP = np.eye(64, dtype=np.float32)[::-1].copy()
    B64 = np.zeros((64, 128), dtype=np.float32)
    for c in range(64):
        B64[c, (c % 2) * 64:(c % 2) * 64 + 64] = 1.0
    R64 = np.zeros((64, 32), dtype=np.float32)
    for c in range(64):
        R64[c, c // 2] = 1.0
    IDT = np.eye(128, dtype=np.float32)
    SC3 = np.array([[1.0], [-1.0], [1.0]], dtype=np.float32)
    return FLIP, B64, R64, IDT, SC3


def _emit_rsqrt(nc, wpool, ss, scale, bias, w=1):
    """ri = 1/sqrt(ss*scale + bias) on DVE only (Quake init + 2 Newton).

    Avoids the Sqrt activation, whose LUT set differs from the gelu set --
    each ACT table-set switch costs ~2.7us on TRN2.  `ss` is [128, w];
    processing several tiles' scalars in one chain amortizes the per-op
    overhead of the 11 tiny DVE ops.
    """
    I32 = mybir.dt.int32
    ms = wpool.tile([128, w], FP, tag="q_ms")
    nc.vector.tensor_scalar(ms[:], ss, scale, bias, OP.mult, OP.add)
    qi = wpool.tile([128, w], I32, tag="q_i")
    nc.vector.tensor_scalar(qi[:], ms[:].bitcast(I32), 1, None,
                            OP.arith_shift_right)
    qj = wpool.tile([128, w], I32, tag="q_j")
    nc.vector.tensor_scalar(qj[:], qi[:], -1, 0x5F3759DF, OP.mult, OP.add)
    y = qj[:].bitcast(FP)
    for it in range(2):
        a = wpool.tile([128, w], FP, tag=f"q_a{it}")
        nc.vector.tensor_mul(a[:], y, y)
        nc.vector.tensor_mul(a[:], a[:], ms[:])
        nc.vector.tensor_scalar(a[:], a[:], -0.5, 1.5, OP.mult, OP.add)
        yn = wpool.tile([128, w], FP, tag=f"q_y{it}")
        nc.vector.tensor_mul(yn[:], y, a[:])
        y = yn[:]
    return y


def build_nc(repeat=1):
    nc = bacc.Bacc("TRN2", target_bir_lowering=False, debug=False)

    # ---- DRAM I/O ----
    seqT_d = nc.dram_tensor("seqT", [D, N], FP, kind="ExternalInput")
    seqrT_d = nc.dram_tensor("seqrT", [D, N // 2], FP, kind="ExternalInput")
    cp128_d = nc.dram_tensor("cp128", [128, 2179], FP, kind="ExternalInput")
    cp64_d = nc.dram_tensor("cp64", [64, 224], FP, kind="ExternalInput")
    cp3_d = nc.dram_tensor("cp3", [3, 2], FP, kind="ExternalInput")
    out_d = nc.dram_tensor("out", [N // 2, D], FP, kind="ExternalOutput")

    with tile.TileContext(nc) as tc:
        with (
            tc.tile_pool(name="const", bufs=1) as cpool,
            tc.tile_pool(name="seq", bufs=1) as spool,
            tc.tile_pool(name="work", bufs=4) as wpool,
            tc.tile_pool(name="big", bufs=2) as bpool,
            tc.tile_pool(name="p512", bufs=3, space="PSUM") as p512,
            tc.tile_pool(name="pmix", bufs=3, space="PSUM") as pmix,
            tc.tile_pool(name="pacc", bufs=1, space="PSUM") as pacc,
        ):
            # ---- constants & weights into SBUF (3 packed DMAs) ----
            cp128 = cpool.tile([128, 2179], FP)
            cp64 = cpool.tile([64, 224], FP)
            cp3 = cpool.tile([3, 2], FP)
            nc.sync.dma_start(cp128[:], cp128_d.ap())
            nc.sync.dma_start(cp64[:], cp64_d.ap())
            nc.sync.dma_start(cp3[:], cp3_d.ap())
            wk = cp128[:, 0:128]
            wv = cp128[:, 128:256]
            wq = cp128[:, 256:384]
            w1 = cp128[:, 384:896]
            w2c = cp128[:, 896:1408]
            w2T = cp128[:, 1408:1920]
            gb = cp128[:, 1920:2048]
            IDT = cp128[:, 2048:2176]
            wh3 = cp128[:, 2176:2179]
            FLIP = cp64[:, 0:64]
            B64 = cp64[:, 64:192]
            R64 = cp64[:, 192:224]
            b3 = cp3[:, 0:1]
            SC3 = cp3[:, 1:2]

            # combined [w_k | w_k - w_v] for the fused k/(k-v) projection
            wkkv = cpool.tile([D, 256], FP)
            nc.vector.tensor_copy(wkkv[:, 0:128], wk)
            nc.vector.tensor_sub(wkkv[:, 128:256], wk, wv)

            # ---- sequence (feature-major) ----
            seqT = spool.tile([D, N], FP)
            for j in range(2):
                nc.sync.dma_start(seqT[:, j * 2048:(j + 1) * 2048],
                                  seqT_d.ap()[:, j * 2048:(j + 1) * 2048])
            seqrT = spool.tile([D, N // 2], FP)
            nc.sync.dma_start(seqrT[:], seqrT_d.ap())

            for _rep in range(repeat):
                # =========================================================
                # Scalar prep: chunk hyperparams -> per-token rho column
                # =========================================================
                reps = cpool.tile([D, NC], FP)     # first token of each chunk
                seqT_c = seqT[:].rearrange("p (c j) -> p c j", j=CHUNK)
                nc.vector.tensor_copy(reps[:], seqT_c[:, :, 0])

                ps_s3 = pmix.tile([3, NC], FP, tag="pmix")
                nc.tensor.matmul(ps_s3[:], wh3, reps[:], start=True, stop=True)
                bias3 = cpool.tile([3, 1], FP)
                nc.vector.tensor_mul(bias3[:], SC3, b3)
                s3 = cpool.tile([3, NC], FP)       # rows: lr, keep, eta
                nc.scalar.activation(s3[:], ps_s3[:], AF.Sigmoid,
                                     bias=bias3[:], scale=SC3)

                # reverse along chunks: s3T = s3^T, s3T_rev = FLIP @ s3T,
                # s3_rev = s3T_rev^T
                ps_s3T = pmix.tile([NC, 3], FP, tag="pmix")
                nc.tensor.transpose(ps_s3T[:], s3[:], IDT[0:3, 0:3])
                s3T = cpool.tile([NC, 3], FP)
                nc.scalar.copy(s3T[:], ps_s3T[:])
                ps_s3Tr = pmix.tile([NC, 3], FP, tag="pmix")
                nc.tensor.matmul(ps_s3Tr[:], FLIP, s3T[:], start=True, stop=True)
                s3Tr = cpool.tile([NC, 3], FP)
                nc.scalar.copy(s3Tr[:], ps_s3Tr[:])
                ps_s3r = pmix.tile([3, NC], FP, tag="pmix")
                nc.tensor.transpose(ps_s3r[:], s3Tr[:], IDT[0:64, 0:64])
                s3r = cpool.tile([3, NC], FP)      # reversed rows: lr_r, keep_r, eta_r
                nc.scalar.copy(s3r[:], ps_s3r[:])

                # shifted operands for the backward-recurrence scans
                kd0 = cpool.tile([1, NC], FP)
                ed0 = cpool.tile([1, NC], FP)
                e0r = cpool.tile([1, NC], FP)
                nc.vector.memset(kd0[:], 0.0)
                nc.vector.memset(ed0[:], 0.0)
                nc.vector.memset(e0r[:], 0.0)
                nc.vector.memset(e0r[:, 0:1], 1.0)
                # rows 1/2 start at partition>0: engines can't, DMA can
                nc.sync.dma_start(kd0[:, 1:NC], s3r[1:2, 0:NC - 1])
                nc.sync.dma_start(ed0[:, 1:NC], s3r[2:3, 0:NC - 1])

                Krev = cpool.tile([1, NC], FP)
                nc.vector.tensor_tensor_scan(Krev[:], kd0[:], e0r[:], 0.0,
                                             OP.mult, OP.add)
                Wrev = cpool.tile([1, NC], FP)
                nc.vector.tensor_tensor_scan(Wrev[:], ed0[:], Krev[:], 0.0,
                                             OP.mult, OP.add)

                rho_rev = cpool.tile([1, NC], FP)
                nc.vector.scalar_tensor_tensor(rho_rev[:], Wrev[:], 2.0 / D,
                                               s3r[0:1, :], OP.mult, OP.mult)
                ps_rr = pmix.tile([NC, 1], FP, tag="pmix")
                nc.tensor.transpose(ps_rr[:], rho_rev[:], IDT[0:1, 0:1])
                rho_rc = cpool.tile([NC, 1], FP)
                nc.scalar.copy(rho_rc[:], ps_rr[:])
                ps_rf = pmix.tile([NC, 1], FP, tag="pmix")
                nc.tensor.matmul(ps_rf[:], FLIP, rho_rc[:], start=True, stop=True)
                rho_fc = cpool.tile([NC, 1], FP)
                nc.scalar.copy(rho_fc[:], ps_rf[:])
                # expand chunks -> per-token per-tile columns: [128 tokens, 32 tiles]
                Bdyn = cpool.tile([64, 128], FP)
                nc.vector.tensor_scalar_mul(Bdyn[:], B64, rho_fc[:])
                ps_rt = pmix.tile([128, 32], FP, tag="pmix")
                nc.tensor.matmul(ps_rt[:], Bdyn[:], R64, start=True, stop=True)
                rho_tok = cpool.tile([128, 32], FP)
                nc.scalar.copy(rho_tok[:], ps_rt[:])

                # =========================================================
                # Gradient chain over 32 token tiles, accumulating
                # dw1 (PSUM), dw2 (PSUM), dg (SBUF via small matmuls)
                # =========================================================
                dw1_acc = pacc.tile([D, HID], FP)      # k^T dz
                dw2_acc = pacc.tile([128, HID], FP)    # a^T dh  (4 chunks side by side)
                dg_sb = cpool.tile([D, 1], FP)
                nc.vector.memset(dg_sb[:], 0.0)

                for p in range(NT // 2):
                    # ---------- phase A: forward for both tiles of the pair ----
                    sspair = wpool.tile([128, 2], FP, tag="sspair")
                    ats, gps, kkks, kvrs, lates, hs = [], [], [], [], [], []
                    for j in range(2):
                        t = 2 * p + j
                        S = seqT[:, t * 128:(t + 1) * 128]

                        ps_kkk = pmix.tile([128, 384], FP, tag="pmix")
                        nc.tensor.matmul(ps_kkk[:, 0:128], wk, S,
                                         start=True, stop=True)
                        nc.tensor.matmul(ps_kkk[:, 128:384], S, wkkv[:],
                                         start=True, stop=True)
                        kkk = wpool.tile([128, 384], FP, tag="kkk")
                        nc.vector.tensor_copy(kkk[:], ps_kkk[:])
                        kf = kkk[:, 0:128]

                        ps_z = p512.tile([128, HID], FP, tag="p512")
                        nc.tensor.matmul(ps_z[:], kf, w1, start=True, stop=True)
                        a_tm = wpool.tile([128, HID], FP, tag="a_tm")
                        nc.scalar.activation(a_tm[:], ps_z[:], AF.Gelu)
                        gp = wpool.tile([128, HID], FP, tag="gp")
                        nc.scalar.activation(gp[:], ps_z[:], AF.Derivative_Gelu)

                        ps_zf = p512.tile([128, HID], FP, tag="p512")
                        for c in range(4):
                            nc.tensor.matmul(ps_zf[:, c * 128:(c + 1) * 128],
                                             w1[:, c * 128:(c + 1) * 128], kf,
                                             start=True, stop=True)
                        af = wpool.tile([128, HID], FP, tag="af")
                        nc.scalar.activation(af[:], ps_zf[:], AF.Gelu)

                        ps_late = pmix.tile([128, 257], FP, tag="pmix")
                        ps_h = ps_late[:, 0:128]
                        for c in range(4):
                            nc.tensor.matmul(ps_h,
                                             af[:, c * 128:(c + 1) * 128],
                                             w2c[:, c * 128:(c + 1) * 128],
                                             start=(c == 0), stop=(c == 3))

                        h_sb = wpool.tile([128, 128], FP, tag=f"h_sb{j}")
                        nc.vector.tensor_copy(h_sb[:], ps_h)
                        scr = wpool.tile([128, 128], FP, tag=f"scr{j}")
                        nc.vector.scalar_tensor_tensor(scr[:], h_sb[:], 1.0,
                                                       h_sb[:], OP.mult, OP.mult,
                                                       accum_out=sspair[:, j:j + 1])
                        kvr = wpool.tile([128, 128], FP, tag="kvr")
                        nc.gpsimd.tensor_scalar_mul(kvr[:], kkk[:, 256:384],
                                                    rho_tok[:, t:t + 1])
                        ats.append(a_tm); gps.append(gp); kkks.append(kkk)
                        kvrs.append(kvr); lates.append(ps_late); hs.append(h_sb)

                    # one rsqrt chain for the pair: ri = rsqrt(ss/D + eps)
                    ri2 = _emit_rsqrt(nc, wpool, sspair[:], 1.0 / D, 1e-6, w=2)
                    rr2 = wpool.tile([128, 2], FP, tag="rr2")
                    nc.vector.tensor_mul(rr2[:], ri2, rho_tok[:, 2 * p:2 * p + 2])

                    # ---------- phase B: backward for both tiles ----------
                    for j in range(2):
                        t = 2 * p + j
                        a_tm, gp, kkk = ats[j], gps[j], kkks[j]
                        kvr, ps_late, h_sb = kvrs[j], lates[j], hs[j]
                        kt = kkk[:, 128:256]
                        ri_c = ri2[:, j:j + 1]

                        # dpred = rho*(h*ri*g + kv)
                        u1 = wpool.tile([128, 128], FP, tag="u1")
                        nc.vector.scalar_tensor_tensor(u1[:], h_sb[:],
                                                       rr2[:, j:j + 1], gb,
                                                       OP.mult, OP.mult)
                        dpred = wpool.tile([128, 128], FP, tag="dpred")
                        nc.gpsimd.tensor_add(dpred[:], u1[:], kvr[:])

                        # dg += sum_t dpred*h*ri (ri folded into matmul rhs)
                        q1 = wpool.tile([128, 128], FP, tag="q1")
                        nc.vector.tensor_mul(q1[:], dpred[:], h_sb[:])
                        ps_dg = ps_late[:, 256:257]
                        nc.tensor.matmul(ps_dg, q1[:], ri_c,
                                         start=True, stop=True)
                        nc.vector.tensor_add(dg_sb[:], dg_sb[:], ps_dg)

                        # dhn = dpred*g ; C2 = sum_d dhn*h ;
                        # dh = dhn*ri - h*C2*ri^3/D
                        dhn = wpool.tile([128, 128], FP, tag="dhn")
                        nc.gpsimd.tensor_mul(dhn[:], dpred[:], gb)
                        cp = wpool.tile([128, 1], FP, tag="cp")
                        scrb = wpool.tile([128, 128], FP, tag="scrb")
                        nc.vector.scalar_tensor_tensor(scrb[:], dhn[:], 1.0,
                                                       h_sb[:], OP.mult, OP.mult,
                                                       accum_out=cp[:])
                        negs = wpool.tile([128, 1], FP, tag="negs")
                        nc.vector.tensor_scalar(negs[:], cp[:], ri_c, -1.0 / D,
                                                OP.mult, OP.mult)
                        s2 = wpool.tile([128, 1], FP, tag="s2")
                        nc.vector.tensor_scalar(s2[:], negs[:], ri_c, ri_c,
                                                OP.mult, OP.mult)
                        t3 = wpool.tile([128, 128], FP, tag="t3")
                        nc.gpsimd.tensor_scalar_mul(t3[:], dhn[:], ri_c)
                        dh = wpool.tile([128, 128], FP, tag="dh")
                        nc.vector.scalar_tensor_tensor(dh[:], h_sb[:], s2[:],
                                                       t3[:], OP.mult, OP.add)

                        # dh^T -> da = dh @ w2^T -> dz = da*gelu'
                        ps_dhT = ps_late[:, 128:256]
                        nc.tensor.transpose(ps_dhT, dh[:], IDT)
                        dhT = wpool.tile([128, 128], FP, tag="dhT")
                        nc.vector.tensor_copy(dhT[:], ps_dhT)

                        ps_da = p512.tile([128, HID], FP, tag="p512")
                        nc.tensor.matmul(ps_da[:], dhT[:], w2T,
                                         start=True, stop=True)
                        dz = bpool.tile([128, HID], FP, tag="dz")
                        nc.vector.tensor_mul(dz[:], ps_da[:], gp[:])

                        nc.tensor.matmul(dw1_acc[:], kt, dz[:],
                                         start=(t == 0), stop=(t == NT - 1))
                        for c in range(4):
                            nc.tensor.matmul(dw2_acc[:, c * 128:(c + 1) * 128],
                                             a_tm[:, c * 128:(c + 1) * 128],
                                             dh[:],
                                             start=(t == 0 and c == 0),
                                             stop=(t == NT - 1 and c == 3))

                # =========================================================
                # Final params: w1_f = -dw1, w2_f = -dw2, g_f = -dg
                # =========================================================
                w1f = cpool.tile([D, HID], FP)
                nc.scalar.activation(w1f[:], dw1_acc[:], AF.Copy, scale=-1.0)
                w2f = cpool.tile([128, HID], FP)
                nc.scalar.activation(w2f[:], dw2_acc[:], AF.Copy, scale=-1.0)

                ps_dgT = pmix.tile([1, 128], FP, tag="pmix")
                nc.tensor.transpose(ps_dgT[:], dg_sb[:], IDT)
                dgT = cpool.tile([1, 128], FP)
                nc.scalar.activation(dgT[:], ps_dgT[:], AF.Copy, scale=-1.0)
                ones_r = cpool.tile([1, 128], FP)
                nc.vector.memset(ones_r[:], 1.0)
                ps_gfb = pmix.tile([128, 128], FP, tag="pmix")
                nc.tensor.matmul(ps_gfb[:], ones_r[:], dgT[:], start=True, stop=True)
                gfb = cpool.tile([128, 128], FP)
                nc.scalar.copy(gfb[:], ps_gfb[:])

                # =========================================================
                # Retrieval on this core's half (16 tiles)
                # =========================================================
                opack = None
                for i in range(NRT):
                    if i % 4 == 0:
                        opack = wpool.tile([128, 512], FP, tag="opack")
                    Sr = seqrT[:, i * 128:(i + 1) * 128]

                    ps_ret = pmix.tile([128, 384], FP, tag="pmix")
                    nc.tensor.matmul(ps_ret[:, 0:128], wq, Sr,
                                     start=True, stop=True)
                    nc.tensor.matmul(ps_ret[:, 128:256], Sr, wq,
                                     start=True, stop=True)
                    qq = wpool.tile([128, 256], FP, tag="kkk")
                    nc.vector.tensor_copy(qq[:], ps_ret[:, 0:256])
                    qf = qq[:, 0:128]
                    qt = qq[:, 128:256]

                    ps_z2 = p512.tile([128, HID], FP, tag="p512")
                    for c in range(4):
                        nc.tensor.matmul(ps_z2[:, c * 128:(c + 1) * 128],
                                         w1f[:, c * 128:(c + 1) * 128], qf,
                                         start=True, stop=True)
                    a2f = wpool.tile([128, HID], FP, tag="af")
                    nc.scalar.activation(a2f[:], ps_z2[:], AF.Gelu)

                    ps_h2 = ps_ret[:, 256:384]
                    for c in range(4):
                        nc.tensor.matmul(ps_h2,
                                         a2f[:, c * 128:(c + 1) * 128],
                                         w2f[:, c * 128:(c + 1) * 128],
                                         start=(c == 0), stop=(c == 3))

                    h2_sb = wpool.tile([128, 128], FP, tag="h_sb0")
                    nc.vector.tensor_copy(h2_sb[:], ps_h2)
                    scr2 = wpool.tile([128, 128], FP, tag="scr0")
                    ss2 = wpool.tile([128, 1], FP, tag="ss")
                    nc.vector.scalar_tensor_tensor(scr2[:], h2_sb[:], 1.0,
                                                   h2_sb[:], OP.mult, OP.mult,
                                                   accum_out=ss2[:])
                    r2i = _emit_rsqrt(nc, wpool, ss2[:], 1.0 / D, 1e-6)
                    hn2 = wpool.tile([128, 128], FP, tag="hn")
                    nc.gpsimd.tensor_scalar_mul(hn2[:], h2_sb[:], r2i)

                    o1 = wpool.tile([128, 128], FP, tag="e1")
                    nc.vector.tensor_mul(o1[:], hn2[:], gfb[:])
                    j = i % 4
                    nc.vector.tensor_add(opack[:, j * 128:(j + 1) * 128],
                                         o1[:], qt)
                    if j == 3:
                        g = i // 4
                        dst = out_d.ap()[g * 512:(g + 1) * 512, :].rearrange(
                            "(j p) d -> p j d", p=128)
                        srcp = opack[:].rearrange("p (j d) -> p j d", d=128)
                        nc.sync.dma_start(dst, srcp)

    nc.compile()
    return nc


def _prep_in_maps(inputs):
    seq = np.ascontiguousarray(inputs["seq"], dtype=np.float32)
    w2 = np.asarray(inputs["w2_0"], dtype=np.float32)
    FLIP, B64, R64, IDT, SC3 = _consts()
    w2c = np.concatenate([w2[128 * c:128 * (c + 1), :] for c in range(4)],
                         axis=1)
    wh3 = np.concatenate(
        [np.asarray(inputs["w_lr"], np.float32),
         np.asarray(inputs["w_decay"], np.float32),
         np.asarray(inputs["w_mom"], np.float32)], axis=1)
    gbm = np.broadcast_to(np.asarray(inputs["gamma0"], np.float32), (128, 128))
    cp128 = np.ascontiguousarray(np.concatenate(
        [np.asarray(inputs["w_k"], np.float32),
         np.asarray(inputs["w_v"], np.float32),
         np.asarray(inputs["w_q"], np.float32),
         np.asarray(inputs["w1_0"], np.float32),
         w2c, w2.T, gbm, IDT, wh3], axis=1))
    cp64 = np.ascontiguousarray(np.concatenate([FLIP, B64, R64], axis=1))
    b3 = np.stack([np.asarray(inputs["b_lr"], np.float32),
                   np.asarray(inputs["b_decay"], np.float32),
                   np.asarray(inputs["b_mom"], np.float32)]).reshape(3, 1)
    cp3 = np.ascontiguousarray(np.concatenate([b3, SC3], axis=1))
    shared = dict(cp128=cp128, cp64=cp64, cp3=cp3)
    in_maps = []
    for c in range(NCORES):
        s, hf = divmod(c, 2)
        x = seq[s]
        m = dict(shared)
        m["seqT"] = np.ascontiguousarray(x.T)
        m["seqrT"] = np.ascontiguousarray(x[hf * 2048:(hf + 1) * 2048].T)
        in_maps.append(m)
    return in_maps


def _get_nc():
    if "nc" not in _CACHED:
        _CACHED["nc"] = build_nc()
    return _CACHED["nc"]


def kernel(**inputs) -> np.ndarray:
    nc = _get_nc()
    in_maps = _prep_in_maps(inputs)
    res = bass_utils.run_bass_kernel_spmd(nc, in_maps,
                                          core_ids=list(range(NCORES)))
    out = np.empty((B, N, D), dtype=np.float32)
    for c in range(NCORES):
        s, hf = divmod(c, 2)
        out[s, hf * 2048:(hf + 1) * 2048] = res.results[c]["out"]
    return out

